# revision 43
# baseline (speedup 1.0000x reference)
"""Trainium2 Bass kernel for nn_MultiHeadAttention (B=4, S=2048, D=512, H=8).

Sharding: tensor-parallel over heads — core c owns head c (Dh=64). Each core
computes q/k/v projections for its head slice (full x replicated, host-pre-
transposed to x^T in bf16), attention for its head over all 4 batches, and
the unnormalized partial out-projection O_c @ Wo[c]; the host divides each
core's partial by its softmax denominators (shipped alongside as a [B,S]
vector), sums the 8 partials, and adds the biases that commute with that
reduction (bo, bv@Wo). All on-core compute is bf16 (fp8 blows the 2e-2
error budget: each fp8-quantized operand alone contributes ~2.5%).

Engine plan (emission order IS the per-engine execution order):
  - PE: projections (W-stationary bf16, batch pairs packed on array column
    halves), row-quadrant-alternating S^T (tile_position (hb*64, 0)) so
    weight loads overlap execution, AV with the ones column of V_aug
    producing softmax denominators in PSUM row 64, out-projection.
  - ACT: exclusively exp(S/8) on [128,1024] tiles (its floor, ~140us).
  - DVE: all PSUM evacuations (no reciprocal/normalize on-core).
  - V reaches its [key, dh] AV layout via DMA XBAR transposes into full
    [128, 80] tiles (strided-slot XBAR destinations are broken on HW);
    row 64 of the padded V^T staging tile carries the ones column.
  - Normalization happens on host; output partial + denominators are bf16.
Batches are paired on SBUF partition halves; pair-1 prep fills PE slack
during pair-0 attention, out-projections fill during pair-1 attention.
"""
import numpy as np

import concourse.bass as bass
import concourse.mybir as mybir
import concourse.tile as tile
from concourse import bacc
from concourse.bass_utils import run_bass_kernel_spmd

B, S, D = 4, 2048, 512
H, DH = 8, 64
NCORES = 8
F32 = mybir.dt.float32
BF16 = mybir.dt.bfloat16
AF = mybir.ActivationFunctionType

NKT = S // 128          # 16 key tiles per batch
NQB = S // 512          # 4 query blocks per batch
NCH = D // 128          # 4 dm chunks

_NC_CACHE = {}


def build_kernel():
    nc = bacc.Bacc("TRN2", target_bir_lowering=False, debug=False)

    xT = nc.dram_tensor("xT", [B, D, S], BF16, kind="ExternalInput")
    # wq|wk|wv (chunk-major, 256 each) | identity (128) packed in one load
    wpack = nc.dram_tensor("wpack", [128, 896], BF16, kind="ExternalInput")
    wo = nc.dram_tensor("wo", [DH, D], BF16, kind="ExternalInput")
    bqk = nc.dram_tensor("bqk", [128, 2], F32, kind="ExternalInput")
    onesin = nc.dram_tensor("onesin", [128, 16, 2], BF16, kind="ExternalInput")
    out = nc.dram_tensor("out", [B * S, D], BF16, kind="ExternalOutput")
    dnm = nc.dram_tensor("dnm", [B, S], BF16, kind="ExternalOutput")

    with tile.TileContext(nc) as tc:
        with (
            tc.tile_pool(name="consts", bufs=1) as consts,
            tc.tile_pool(name="xtp", bufs=16) as xtp,
            tc.tile_pool(name="qkp", bufs=2) as qkp,
            tc.tile_pool(name="vtp", bufs=4) as vtp,
            tc.tile_pool(name="vp", bufs=4) as vp,
            tc.tile_pool(name="ptp", bufs=3) as ptp,
            tc.tile_pool(name="otp", bufs=3) as otp,
            tc.tile_pool(name="sop", bufs=4) as sopp,
            tc.tile_pool(name="psA", bufs=2, space="PSUM") as psA,   # pst [128,1024] f32
            tc.tile_pool(name="psO", bufs=2, space="PSUM") as psO,   # po [65,512] f32
            tc.tile_pool(name="psM", bufs=2, space="PSUM") as psM,   # misc [128,512] f32
        ):
            bqk_sb = consts.tile([128, 2], F32)
            wp_sb = consts.tile([128, 896], BF16)
            wo_sb = consts.tile([DH, D], BF16)
            warm = consts.tile([128, 1], BF16)
            nc.sync.dma_start(out=wp_sb[:], in_=wpack[:])
            nc.scalar.dma_start(out=bqk_sb[:], in_=bqk[:])
            # warmup: pulls the Exp table load (~1.3us) into the kernel head
            nc.scalar.activation(warm[:], bqk_sb[:, 0:1], AF.Exp, scale=0.125)
            nc.gpsimd.dma_start(out=wo_sb[:], in_=wo[:])
            bq_sb = bqk_sb[:, 0:1]
            bk_sb = bqk_sb[:, 1:2]
            ident = wp_sb[:, 768:896]

            def w_sl(kind, ci):
                off = {"q": 0, "k": 256, "v": 512}[kind] + ci * DH
                return wp_sb[:, bass.ds(off, DH)]

            state = {}

            def alloc_pair(pr):
                st = {"xt": {}, "vt": {}, "v": {}, "ot": {}}
                st["qt"] = qkp.tile([128, S], BF16, tag="qt", name=f"qt_{pr}")
                st["kt"] = qkp.tile([128, S], BF16, tag="kt", name=f"kt_{pr}")
                for half in range(2):
                    b = pr * 2 + half
                    v_b = vp.tile([128, NKT, DH + 2], BF16, tag="v", name=f"v_{b}")
                    nc.gpsimd.dma_start(out=v_b[:, :, DH:DH + 2], in_=onesin[:])
                    st["v"][half] = v_b
                state[pr] = st

            def emit_xt_loads(pr):
                st = state[pr]
                for half in range(2):
                    b = pr * 2 + half
                    st["xt"][half] = [
                        xtp.tile([128, S], BF16, tag="xt", name=f"xt_{b}_{ci}")
                        for ci in range(NCH)
                    ]
                # blk0 fine-grained (unblocks prep(0) fast), rest coarse —
                # per-DMA issue on the queues is ~0.6us, so fewer, larger
                # transfers keep the head off the issue-rate wall
                engs3 = [nc.sync, nc.gpsimd, nc.scalar]
                q = 0
                for ci in range(NCH):
                    for half in range(2):
                        if pr == 0:
                            eng = engs3[q % 3]
                        else:
                            eng = engs3[q % 2]
                        q += 1
                        eng.dma_start(
                            out=st["xt"][half][ci][:, bass.ts(0, 512)],
                            in_=xT[pr * 2 + half, bass.ts(ci, 128), bass.ts(0, 512)],
                        )
                for ci in range(NCH):
                    for half in range(2):
                        eng = nc.sync if ((ci + half) % 2 == 0) else nc.gpsimd
                        eng.dma_start(
                            out=st["xt"][half][ci][:, bass.ds(512, 1536)],
                            in_=xT[pr * 2 + half, bass.ts(ci, 128), bass.ds(512, 1536)],
                        )

            def emit_prep_q(pr, blk):
                st = state[pr]
                sl = bass.ts(blk, 512)
                pq = psM.tile([128, 512], F32, tag="psM", name=f"pq_{pr}_{blk}")
                for ci in range(NCH):
                    for half in range(2):
                        nc.tensor.matmul(
                            pq[half * DH:(half + 1) * DH, :],
                            w_sl("q", ci), st["xt"][half][ci][:, sl],
                            start=(ci == 0), stop=(ci == NCH - 1),
                            tile_position=(0, half * DH),
                        )
                nc.vector.tensor_scalar_add(st["qt"][:, sl], pq[:], bq_sb)

            def emit_prep_k(pr, blk):
                st = state[pr]
                sl = bass.ts(blk, 512)
                pk = psM.tile([128, 512], F32, tag="psM", name=f"pk_{pr}_{blk}")
                for ci in range(NCH):
                    for half in range(2):
                        nc.tensor.matmul(
                            pk[half * DH:(half + 1) * DH, :],
                            w_sl("k", ci), st["xt"][half][ci][:, sl],
                            start=(ci == 0), stop=(ci == NCH - 1),
                            tile_position=(0, half * DH),
                        )
                nc.vector.tensor_scalar_add(st["kt"][:, sl], pk[:], bk_sb)

            def emit_prep_v(pr, blk):
                st = state[pr]
                sl = bass.ts(blk, 512)
                if blk == 0:
                    for half in range(2):
                        b = pr * 2 + half
                        vt_b = vtp.tile([DH, S], BF16, tag="vt", name=f"vt_{b}")
                        st["vt"][half] = vt_b
                pv = psM.tile([128, 512], F32, tag="psM", name=f"pv_{pr}_{blk}")
                for ci in range(NCH):
                    for half in range(2):
                        nc.tensor.matmul(
                            pv[half * DH:(half + 1) * DH, :],
                            w_sl("v", ci), st["xt"][half][ci][:, sl],
                            start=(ci == 0), stop=(ci == NCH - 1),
                            tile_position=(0, half * DH),
                        )
                nc.vector.tensor_copy(st["vt"][0][:, sl], pv[0:DH, :])
                nc.vector.tensor_copy(st["vt"][1][:, sl], pv[DH:128, :])
                for half in range(2):
                    b = pr * 2 + half
                    pvtr = psM.tile([128, 256], BF16, tag="psM", name=f"pvtr_{b}_{blk}")
                    for j in range(4):
                        nc.tensor.transpose(
                            pvtr[:, bass.ts(j, 64)],
                            st["vt"][half][:, bass.ds(blk * 512 + j * 128, 128)],
                            wp_sb[0:DH, bass.ds(768, DH)],
                        )
                    nc.vector.tensor_copy(
                        st["v"][half][:, bass.ds(blk * 4, 4), 0:DH],
                        pvtr[:].rearrange("p (k m) -> p k m", m=64),
                    )

            # --- software-pipelined attention over a flat (pr, qq, kt) stream:
            # per step i the PE does [S^T(i+1), filler, AV(i-1)], so AV never
            # waits on its exp (which completed during the previous step) and
            # the PE stays continuously busy (p-state ramp to 2.4 GHz).
            psts = {}
            ptts = {}
            pos = {}

            def emit_st(pr, qq, kt_i, i):
                st = state[pr]
                pst = psA.tile([128, 1024], F32, tag="psA", name=f"pst_{pr}_{qq}_{kt_i}")
                for hb in range(2):
                    nc.tensor.matmul(
                        pst[:, bass.ts(hb, 512)],
                        st["kt"][hb * DH:(hb + 1) * DH, bass.ts(kt_i, 128)],
                        st["qt"][hb * DH:(hb + 1) * DH, bass.ts(qq, 512)],
                        start=True, stop=True,
                        tile_position=(hb * DH, 0),
                    )
                psts[i] = pst

            def emit_exp(i):
                ptt = ptp.tile([128, 1024], BF16, tag="pt", name=f"ptt_{i}")
                nc.scalar.activation(ptt[:], psts.pop(i)[:], AF.Exp, scale=0.125)
                ptts[i] = ptt

            def emit_av(pr, qq, kt_i, i):
                st = state[pr]
                if kt_i == 0:
                    pos[(pr, qq)] = [
                        psO.tile([DH + 1, 512], F32, tag="psO", name=f"po{hb}_{pr}_{qq}")
                        for hb in range(2)
                    ]
                po = pos[(pr, qq)]
                ptt = ptts.pop(i)
                for hb in range(2):
                    nc.tensor.matmul(
                        po[hb][:],
                        st["v"][hb][:, kt_i, 0:DH + 1],
                        ptt[:, bass.ts(hb, 512)],
                        start=(kt_i == 0), stop=(kt_i == NKT - 1),
                    )

            def emit_po_evac(pr, qq):
                st = state[pr]
                if qq == 0:
                    for half in range(2):
                        st["ot"][half] = otp.tile(
                            [DH + 1, S], BF16, tag="ot", name=f"ot_{pr * 2 + half}"
                        )
                po = pos.pop((pr, qq))
                for hb in range(2):
                    nc.vector.tensor_copy(st["ot"][hb][:, bass.ts(qq, 512)], po[hb][:])

            def emit_op_tt(pr, half, tt):
                st = state[pr]
                b = pr * 2 + half
                ot_b = st["ot"][half]
                pop = psM.tile([128, 512], F32, tag="psM", name=f"pop_{b}_{tt}")
                nc.tensor.matmul(
                    pop[:], ot_b[0:DH, bass.ts(tt, 128)], wo_sb[:],
                    start=True, stop=True,
                )
                so = sopp.tile([128, 512], BF16, tag="so", name=f"so_{b}_{tt}")
                nc.vector.tensor_copy(so[:], pop[:])
                eng = nc.gpsimd if (b * NKT + tt) % 2 == 0 else nc.sync
                eng.dma_start(
                    out=out[bass.ds(b * S + tt * 128, 128), :], in_=so[:]
                )

            def emit_dnm_dma(pr, half):
                b = pr * 2 + half
                nc.gpsimd.dma_start(
                    out=dnm[b:b + 1, :], in_=state[pr]["ot"][half][DH:DH + 1, :]
                )

            # ---------------- emission schedule ----------------
            import functools
            P = functools.partial
            alloc_pair(0)
            alloc_pair(1)
            emit_xt_loads(0)
            emit_xt_loads(1)

            # minimal pair-0 head: block 0 of q/k/v (+ first 8 V transposes)
            emit_prep_q(0, 0)
            emit_prep_k(0, 0)
            emit_prep_v(0, 0)

            # fillers staged by earliest-allowed step so a filler whose DMA
            # hasn't landed can't convoy the in-order PE queue
            fill = []
            ms = 0
            for blk in (1, 2, 3):
                fill.append((ms, P(emit_prep_k, 0, blk))); ms += 1
                fill.append((ms, P(emit_prep_v, 0, blk))); ms += 1
                fill.append((ms, P(emit_prep_q, 0, blk))); ms += 1
            ms = 16
            for blk in range(NQB):
                fill.append((ms, P(emit_prep_k, 1, blk))); ms += 2
                fill.append((ms, P(emit_prep_v, 1, blk))); ms += 2
                fill.append((ms, P(emit_prep_q, 1, blk))); ms += 2

            units = [(pr, qq, kt) for pr in range(2) for qq in range(NQB)
                     for kt in range(NKT)]
            NSTEP = len(units)
            emit_st(*units[0], 0)
            for i in range(NSTEP):
                emit_exp(i)
                if i + 1 < NSTEP:
                    emit_st(*units[i + 1], i + 1)
                if fill and fill[0][0] <= i:
                    fill.pop(0)[1]()
                if i >= 1:
                    pr, qq, kt = units[i - 1]
                    emit_av(pr, qq, kt, i - 1)
                    if kt == NKT - 1:
                        emit_po_evac(pr, qq)
                        for half in range(2):
                            for tt in range(qq * 4, qq * 4 + 4):
                                fill.append((0, P(emit_op_tt, pr, half, tt)))
                        if qq == NQB - 1:
                            for half in range(2):
                                fill.append((0, P(emit_dnm_dma, pr, half)))
            pr, qq, kt = units[NSTEP - 1]
            emit_av(pr, qq, kt, NSTEP - 1)
            while fill:
                fill.pop(0)[1]()
            # tail: fine-grained evac of the last qq so each out-projection
            # starts as soon as its 128-token slice of O^T lands
            st = state[pr]
            po = pos.pop((pr, qq))
            for tt_rel in range(4):
                tt = qq * 4 + tt_rel
                dsl = bass.ds(qq * 512 + tt_rel * 128, 128)
                for hb in range(2):
                    nc.vector.tensor_copy(
                        st["ot"][hb][:, dsl], po[hb][:, bass.ts(tt_rel, 128)]
                    )
                for hb in range(2):
                    emit_op_tt(pr, hb, tt)
            for half in range(2):
                emit_dnm_dma(pr, half)

    nc.compile()
    return nc


def kernel(x, Wq, bq, Wk, bk, Wv, bv, Wo, bo):
    import ml_dtypes
    BF = ml_dtypes.bfloat16
    x = np.asarray(x, dtype=np.float32)
    xT = np.ascontiguousarray(np.transpose(x, (0, 2, 1))).astype(BF)
    Wq = np.asarray(Wq, dtype=np.float32)
    Wk = np.asarray(Wk, dtype=np.float32)
    Wv = np.asarray(Wv, dtype=np.float32)
    Wo = np.asarray(Wo, dtype=np.float32)
    bq = np.asarray(bq, dtype=np.float32)
    bk = np.asarray(bk, dtype=np.float32)
    bv = np.asarray(bv, dtype=np.float32)
    bo = np.asarray(bo, dtype=np.float32)

    if "nc" not in _NC_CACHE:
        _NC_CACHE["nc"] = build_kernel()
    nc = _NC_CACHE["nc"]

    eye = np.eye(128, dtype=np.float32)
    ones = np.zeros((128, 16, 2), dtype=BF)
    ones[:, :, 0] = 1.0

    def cmajor(W, hs):
        # [p, c*64+m] = W[c*128+p, hs][m]
        return W[:, hs].reshape(4, 128, DH).transpose(1, 0, 2).reshape(128, 4 * DH)

    in_maps = []
    for c in range(NCORES):
        hs = slice(c * DH, (c + 1) * DH)
        wp = np.concatenate(
            [cmajor(Wq, hs), cmajor(Wk, hs), cmajor(Wv, hs), eye], axis=1
        )
        in_maps.append({
            "xT": xT,
            "wpack": np.ascontiguousarray(wp).astype(BF),
            "wo": np.ascontiguousarray(Wo[hs, :]).astype(BF),
            "bqk": np.ascontiguousarray(
                np.stack([np.concatenate([bq[hs], bq[hs]]),
                          np.concatenate([bk[hs], bk[hs]])], axis=1)),
            "onesin": ones,
        })

    res = run_bass_kernel_spmd(nc, in_maps, list(range(NCORES)))

    acc = np.zeros((B * S, D), dtype=np.float32)
    for c in range(NCORES):
        o = np.asarray(res.results[c]["out"]).astype(np.float32)
        d = np.asarray(res.results[c]["dnm"]).astype(np.float32)
        acc += o / d.reshape(B * S, 1)
    # biases that commute with the head-reduction, applied at gather time
    acc += bo[None, :] + (bv @ Wo)[None, :]
    return acc.reshape(B, S, D)


# revision 44
# speedup vs baseline: 1.0192x; 1.0192x over previous
"""Trainium2 Bass kernel for nn_MultiHeadAttention (B=4, S=2048, D=512, H=8).

Sharding: tensor-parallel over heads — core c owns head c (Dh=64). Each core
computes q/k/v projections for its head slice (full x replicated, host-pre-
transposed to x^T in bf16), attention for its head over all 4 batches, and
the unnormalized partial out-projection O_c @ Wo[c]; the host divides each
core's partial by its softmax denominators (shipped alongside as a [B,S]
vector), sums the 8 partials, and adds the biases that commute with that
reduction (bo, bv@Wo). All on-core compute is bf16 (fp8 blows the 2e-2
error budget: each fp8-quantized operand alone contributes ~2.5%).

Engine plan (emission order IS the per-engine execution order):
  - PE: projections (W-stationary bf16, batch pairs packed on array column
    halves), row-quadrant-alternating S^T (tile_position (hb*64, 0)) so
    weight loads overlap execution, AV with the ones column of V_aug
    producing softmax denominators in PSUM row 64, out-projection.
  - ACT: exclusively exp(S/8) on [128,1024] tiles (its floor, ~140us).
  - DVE: all PSUM evacuations (no reciprocal/normalize on-core).
  - V reaches its [key, dh] AV layout via DMA XBAR transposes into full
    [128, 80] tiles (strided-slot XBAR destinations are broken on HW);
    row 64 of the padded V^T staging tile carries the ones column.
  - Normalization happens on host; output partial + denominators are bf16.
Batches are paired on SBUF partition halves; pair-1 prep fills PE slack
during pair-0 attention, out-projections fill during pair-1 attention.
"""
import numpy as np

import concourse.bass as bass
import concourse.mybir as mybir
import concourse.tile as tile
from concourse import bacc
from concourse.bass_utils import run_bass_kernel_spmd

B, S, D = 4, 2048, 512
H, DH = 8, 64
NCORES = 8
F32 = mybir.dt.float32
BF16 = mybir.dt.bfloat16
AF = mybir.ActivationFunctionType

NKT = S // 128          # 16 key tiles per batch
NQB = S // 512          # 4 query blocks per batch
NCH = D // 128          # 4 dm chunks

_NC_CACHE = {}


def build_kernel():
    nc = bacc.Bacc("TRN2", target_bir_lowering=False, debug=False)

    xT = nc.dram_tensor("xT", [B, D, S], BF16, kind="ExternalInput")
    # wq|wk|wv (chunk-major, 256 each) | identity (128) packed in one load
    wpack = nc.dram_tensor("wpack", [128, 896], BF16, kind="ExternalInput")
    wo = nc.dram_tensor("wo", [DH, D], BF16, kind="ExternalInput")
    bqk = nc.dram_tensor("bqk", [128, 2], F32, kind="ExternalInput")
    onesin = nc.dram_tensor("onesin", [128, 16, 2], BF16, kind="ExternalInput")
    out = nc.dram_tensor("out", [B * S, D], BF16, kind="ExternalOutput")
    dnm = nc.dram_tensor("dnm", [B, S], BF16, kind="ExternalOutput")

    with tile.TileContext(nc) as tc:
        with (
            tc.tile_pool(name="consts", bufs=1) as consts,
            tc.tile_pool(name="xtp", bufs=16) as xtp,
            tc.tile_pool(name="qkp", bufs=2) as qkp,
            tc.tile_pool(name="vtp", bufs=4) as vtp,
            tc.tile_pool(name="vp", bufs=4) as vp,
            tc.tile_pool(name="ptp", bufs=3) as ptp,
            tc.tile_pool(name="otp", bufs=3) as otp,
            tc.tile_pool(name="sop", bufs=4) as sopp,
            tc.tile_pool(name="psA", bufs=2, space="PSUM") as psA,   # pst [128,1024] f32
            tc.tile_pool(name="psO", bufs=2, space="PSUM") as psO,   # po [65,512] f32
            tc.tile_pool(name="psM", bufs=2, space="PSUM") as psM,   # misc [128,512] f32
        ):
            bqk_sb = consts.tile([128, 2], F32)
            wp_sb = consts.tile([128, 896], BF16)
            wo_sb = consts.tile([DH, D], BF16)
            warm = consts.tile([128, 1], BF16)
            nc.sync.dma_start(out=wp_sb[:], in_=wpack[:])
            nc.scalar.dma_start(out=bqk_sb[:], in_=bqk[:])
            # warmup: pulls the Exp table load (~1.3us) into the kernel head
            nc.scalar.activation(warm[:], bqk_sb[:, 0:1], AF.Exp, scale=0.125)
            nc.gpsimd.dma_start(out=wo_sb[:], in_=wo[:])
            bq_sb = bqk_sb[:, 0:1]
            bk_sb = bqk_sb[:, 1:2]
            ident = wp_sb[:, 768:896]

            def w_sl(kind, ci):
                off = {"q": 0, "k": 256, "v": 512}[kind] + ci * DH
                return wp_sb[:, bass.ds(off, DH)]

            state = {}

            def alloc_pair(pr):
                st = {"xt": {}, "vt": {}, "v": {}, "ot": {}}
                st["qt"] = qkp.tile([128, S], BF16, tag="qt", name=f"qt_{pr}")
                st["kt"] = qkp.tile([128, S], BF16, tag="kt", name=f"kt_{pr}")
                for half in range(2):
                    b = pr * 2 + half
                    v_b = vp.tile([128, NKT, DH + 2], BF16, tag="v", name=f"v_{b}")
                    nc.gpsimd.dma_start(out=v_b[:, :, DH:DH + 2], in_=onesin[:])
                    st["v"][half] = v_b
                state[pr] = st

            def emit_xt_loads(pr):
                st = state[pr]
                for half in range(2):
                    b = pr * 2 + half
                    st["xt"][half] = [
                        xtp.tile([128, S], BF16, tag="xt", name=f"xt_{b}_{ci}")
                        for ci in range(NCH)
                    ]
                # blk-major, halves interleaved: prep(blk) needs both halves
                engs3 = [nc.sync, nc.gpsimd, nc.scalar]
                q = 0
                for blk in range(NQB):
                    for ci in range(NCH):
                        for half in range(2):
                            if pr == 0 and blk == 0:
                                eng = engs3[q % 3]
                                q += 1
                            else:
                                eng = nc.sync if ((ci + half) % 2 == 0) else nc.gpsimd
                            eng.dma_start(
                                out=st["xt"][half][ci][:, bass.ts(blk, 512)],
                                in_=xT[pr * 2 + half, bass.ts(ci, 128), bass.ts(blk, 512)],
                            )

            def emit_prep_q(pr, blk):
                st = state[pr]
                sl = bass.ts(blk, 512)
                pq = psM.tile([128, 512], F32, tag="psM", name=f"pq_{pr}_{blk}")
                for ci in range(NCH):
                    for half in range(2):
                        nc.tensor.matmul(
                            pq[half * DH:(half + 1) * DH, :],
                            w_sl("q", ci), st["xt"][half][ci][:, sl],
                            start=(ci == 0), stop=(ci == NCH - 1),
                            tile_position=(0, half * DH),
                        )
                nc.vector.tensor_scalar_add(st["qt"][:, sl], pq[:], bq_sb)

            def emit_prep_k(pr, blk):
                st = state[pr]
                sl = bass.ts(blk, 512)
                pk = psM.tile([128, 512], F32, tag="psM", name=f"pk_{pr}_{blk}")
                for ci in range(NCH):
                    for half in range(2):
                        nc.tensor.matmul(
                            pk[half * DH:(half + 1) * DH, :],
                            w_sl("k", ci), st["xt"][half][ci][:, sl],
                            start=(ci == 0), stop=(ci == NCH - 1),
                            tile_position=(0, half * DH),
                        )
                nc.vector.tensor_scalar_add(st["kt"][:, sl], pk[:], bk_sb)

            def emit_prep_v(pr, blk):
                st = state[pr]
                sl = bass.ts(blk, 512)
                if blk == 0:
                    for half in range(2):
                        b = pr * 2 + half
                        vt_b = vtp.tile([DH, S], BF16, tag="vt", name=f"vt_{b}")
                        st["vt"][half] = vt_b
                pv = psM.tile([128, 512], F32, tag="psM", name=f"pv_{pr}_{blk}")
                for ci in range(NCH):
                    for half in range(2):
                        nc.tensor.matmul(
                            pv[half * DH:(half + 1) * DH, :],
                            w_sl("v", ci), st["xt"][half][ci][:, sl],
                            start=(ci == 0), stop=(ci == NCH - 1),
                            tile_position=(0, half * DH),
                        )
                nc.vector.tensor_copy(st["vt"][0][:, sl], pv[0:DH, :])
                nc.vector.tensor_copy(st["vt"][1][:, sl], pv[DH:128, :])
                for half in range(2):
                    b = pr * 2 + half
                    pvtr = psM.tile([128, 256], BF16, tag="psM", name=f"pvtr_{b}_{blk}")
                    for j in range(4):
                        nc.tensor.transpose(
                            pvtr[:, bass.ts(j, 64)],
                            st["vt"][half][:, bass.ds(blk * 512 + j * 128, 128)],
                            wp_sb[0:DH, bass.ds(768, DH)],
                        )
                    nc.vector.tensor_copy(
                        st["v"][half][:, bass.ds(blk * 4, 4), 0:DH],
                        pvtr[:].rearrange("p (k m) -> p k m", m=64),
                    )

            # --- software-pipelined attention over a flat (pr, qq, kt) stream:
            # per step i the PE does [S^T(i+1), filler, AV(i-1)], so AV never
            # waits on its exp (which completed during the previous step) and
            # the PE stays continuously busy (p-state ramp to 2.4 GHz).
            psts = {}
            ptts = {}
            pos = {}

            def emit_st(pr, qq, kt_i, i):
                st = state[pr]
                pst = psA.tile([128, 1024], F32, tag="psA", name=f"pst_{pr}_{qq}_{kt_i}")
                for hb in range(2):
                    nc.tensor.matmul(
                        pst[:, bass.ts(hb, 512)],
                        st["kt"][hb * DH:(hb + 1) * DH, bass.ts(kt_i, 128)],
                        st["qt"][hb * DH:(hb + 1) * DH, bass.ts(qq, 512)],
                        start=True, stop=True,
                        tile_position=(hb * DH, 0),
                    )
                psts[i] = pst

            def emit_exp(i):
                ptt = ptp.tile([128, 1024], BF16, tag="pt", name=f"ptt_{i}")
                nc.scalar.activation(ptt[:], psts.pop(i)[:], AF.Exp, scale=0.125)
                ptts[i] = ptt

            def emit_av(pr, qq, kt_i, i):
                st = state[pr]
                if kt_i == 0:
                    pos[(pr, qq)] = [
                        psO.tile([DH + 1, 512], F32, tag="psO", name=f"po{hb}_{pr}_{qq}")
                        for hb in range(2)
                    ]
                po = pos[(pr, qq)]
                ptt = ptts.pop(i)
                for hb in range(2):
                    nc.tensor.matmul(
                        po[hb][:],
                        st["v"][hb][:, kt_i, 0:DH + 1],
                        ptt[:, bass.ts(hb, 512)],
                        start=(kt_i == 0), stop=(kt_i == NKT - 1),
                    )

            def emit_po_evac(pr, qq):
                st = state[pr]
                if qq == 0:
                    for half in range(2):
                        st["ot"][half] = otp.tile(
                            [DH + 1, S], BF16, tag="ot", name=f"ot_{pr * 2 + half}"
                        )
                po = pos.pop((pr, qq))
                for hb in range(2):
                    nc.vector.tensor_copy(st["ot"][hb][:, bass.ts(qq, 512)], po[hb][:])

            def emit_op_tt(pr, half, tt):
                st = state[pr]
                b = pr * 2 + half
                ot_b = st["ot"][half]
                pop = psM.tile([128, 512], F32, tag="psM", name=f"pop_{b}_{tt}")
                nc.tensor.matmul(
                    pop[:], ot_b[0:DH, bass.ts(tt, 128)], wo_sb[:],
                    start=True, stop=True,
                )
                so = sopp.tile([128, 512], BF16, tag="so", name=f"so_{b}_{tt}")
                nc.vector.tensor_copy(so[:], pop[:])
                eng = nc.gpsimd if (b * NKT + tt) % 2 == 0 else nc.sync
                eng.dma_start(
                    out=out[bass.ds(b * S + tt * 128, 128), :], in_=so[:]
                )

            def emit_dnm_dma(pr, half):
                b = pr * 2 + half
                nc.gpsimd.dma_start(
                    out=dnm[b:b + 1, :], in_=state[pr]["ot"][half][DH:DH + 1, :]
                )

            # ---------------- emission schedule ----------------
            import functools
            P = functools.partial
            alloc_pair(0)
            alloc_pair(1)
            emit_xt_loads(0)
            emit_xt_loads(1)

            # minimal pair-0 head: block 0 of q/k/v (+ first 8 V transposes)
            emit_prep_q(0, 0)
            emit_prep_k(0, 0)
            emit_prep_v(0, 0)

            # fillers staged by earliest-allowed step so a filler whose DMA
            # hasn't landed can't convoy the in-order PE queue
            fill = []
            ms = 0
            for blk in (1, 2, 3):
                fill.append((ms, P(emit_prep_k, 0, blk))); ms += 1
                fill.append((ms, P(emit_prep_v, 0, blk))); ms += 1
                fill.append((ms, P(emit_prep_q, 0, blk))); ms += 1
            ms = 16
            for blk in range(NQB):
                fill.append((ms, P(emit_prep_k, 1, blk))); ms += 2
                fill.append((ms, P(emit_prep_v, 1, blk))); ms += 2
                fill.append((ms, P(emit_prep_q, 1, blk))); ms += 2

            units = [(pr, qq, kt) for pr in range(2) for qq in range(NQB)
                     for kt in range(NKT)]
            NSTEP = len(units)
            emit_st(*units[0], 0)
            for i in range(NSTEP):
                emit_exp(i)
                if i + 1 < NSTEP:
                    emit_st(*units[i + 1], i + 1)
                if fill and fill[0][0] <= i:
                    fill.pop(0)[1]()
                if i >= 1:
                    pr, qq, kt = units[i - 1]
                    emit_av(pr, qq, kt, i - 1)
                    if kt == NKT - 1:
                        emit_po_evac(pr, qq)
                        for half in range(2):
                            for tt in range(qq * 4, qq * 4 + 4):
                                fill.append((0, P(emit_op_tt, pr, half, tt)))
                        if qq == NQB - 1:
                            for half in range(2):
                                fill.append((0, P(emit_dnm_dma, pr, half)))
            pr, qq, kt = units[NSTEP - 1]
            emit_av(pr, qq, kt, NSTEP - 1)
            while fill:
                fill.pop(0)[1]()
            # tail: fine-grained evac of the last qq so each out-projection
            # starts as soon as its 128-token slice of O^T lands
            st = state[pr]
            po = pos.pop((pr, qq))
            for tt_rel in range(4):
                tt = qq * 4 + tt_rel
                dsl = bass.ds(qq * 512 + tt_rel * 128, 128)
                for hb in range(2):
                    nc.vector.tensor_copy(
                        st["ot"][hb][:, dsl], po[hb][:, bass.ts(tt_rel, 128)]
                    )
                for hb in range(2):
                    emit_op_tt(pr, hb, tt)
            for half in range(2):
                emit_dnm_dma(pr, half)

    nc.compile()
    return nc


def kernel(x, Wq, bq, Wk, bk, Wv, bv, Wo, bo):
    import ml_dtypes
    BF = ml_dtypes.bfloat16
    x = np.asarray(x, dtype=np.float32)
    xT = np.ascontiguousarray(np.transpose(x, (0, 2, 1))).astype(BF)
    Wq = np.asarray(Wq, dtype=np.float32)
    Wk = np.asarray(Wk, dtype=np.float32)
    Wv = np.asarray(Wv, dtype=np.float32)
    Wo = np.asarray(Wo, dtype=np.float32)
    bq = np.asarray(bq, dtype=np.float32)
    bk = np.asarray(bk, dtype=np.float32)
    bv = np.asarray(bv, dtype=np.float32)
    bo = np.asarray(bo, dtype=np.float32)

    if "nc" not in _NC_CACHE:
        _NC_CACHE["nc"] = build_kernel()
    nc = _NC_CACHE["nc"]

    eye = np.eye(128, dtype=np.float32)
    ones = np.zeros((128, 16, 2), dtype=BF)
    ones[:, :, 0] = 1.0

    def cmajor(W, hs):
        # [p, c*64+m] = W[c*128+p, hs][m]
        return W[:, hs].reshape(4, 128, DH).transpose(1, 0, 2).reshape(128, 4 * DH)

    in_maps = []
    for c in range(NCORES):
        hs = slice(c * DH, (c + 1) * DH)
        wp = np.concatenate(
            [cmajor(Wq, hs), cmajor(Wk, hs), cmajor(Wv, hs), eye], axis=1
        )
        in_maps.append({
            "xT": xT,
            "wpack": np.ascontiguousarray(wp).astype(BF),
            "wo": np.ascontiguousarray(Wo[hs, :]).astype(BF),
            "bqk": np.ascontiguousarray(
                np.stack([np.concatenate([bq[hs], bq[hs]]),
                          np.concatenate([bk[hs], bk[hs]])], axis=1)),
            "onesin": ones,
        })

    res = run_bass_kernel_spmd(nc, in_maps, list(range(NCORES)))

    acc = np.zeros((B * S, D), dtype=np.float32)
    for c in range(NCORES):
        o = np.asarray(res.results[c]["out"]).astype(np.float32)
        d = np.asarray(res.results[c]["dnm"]).astype(np.float32)
        acc += o / d.reshape(B * S, 1)
    # biases that commute with the head-reduction, applied at gather time
    acc += bo[None, :] + (bv @ Wo)[None, :]
    return acc.reshape(B, S, D)
